# revision 38
# baseline (speedup 1.0000x reference)
"""Trainium2 Bass kernel for nn_BaselineSpanScorer (span-pair MLP scorer), v3.

reference:
    xs        [32, 512, 1024] f32
    spans     [65536, 2] int   (begin/end token index within sequence)
    batch_ids [65536] int
    W1 [2048, 150], b1 [150], W2 [150, 17], b2 [17]
    out[n] = relu(concat(xs[b, s0], xs[b, s1]) @ W1 + b1) @ W2 + b2

Strategy (8 NeuronCores, data parallel, per-core graph is input-shape fixed):
  - Shard xs by batch: core c owns batches [4c, 4c+4) = 2048 token rows.
  - Factorize: A[t] = xs[t] @ W1[:1024], G[t] = xs[t] @ W1[1024:], so
    pre[n] = A[i0_n] + G[i1_n] + b1 (token reuse cuts matmul work ~4x).
  - Stage 1: 16 token tiles x 8 K-blocks of fused [A|G] 300-col fp16
    matmuls. A tiles stay in SBUF (one-hot lhsT) and are also DMAed to a
    DRAM table (for the overflow gather); G tiles go to a DRAM table.
  - Spans are bucketed by i0 token block (16 buckets of 512 + 1 overflow
    tile): a block tile's one-hot contraction needs exactly ONE 128-token
    window. Overflow spans use a gathered A side instead.
  - G side (and overflow A side) via DRAM-source transpose dma_gather
    (features land on partitions), 4 SWDGE queues.
  - Stage 2 per span tile: PSUM accumulates onehot-A (or identity@gatherA)
    + identity@gatherG on the PE; hi-22 features add on DVE; relu+bias on
    ACT; [150]x[17] W2 contraction; bias; DMA out scores^T.
  - Host scatters per-core outputs back to original span order.

Compute dtype fp16 (rel err ~1e-3 vs f32 reference), f32 output.
"""

import os

os.environ.setdefault("MYCRO_LOCAL_CACHE", "1")

import numpy as np

# ---------------- problem constants (hardcoded per spec) ----------------
B, T, D = 32, 512, 1024
N_SPANS = 65536
H, L = 150, 17
NCORES = 8
BPC = B // NCORES        # batches per core = 4
TC = BPC * T             # tokens per core = 2048
N_KB = D // 128          # K blocks in stage 1 = 8
N_TT = TC // 128         # token tiles in stage 1 = 16
N_CH = 8                 # xsT load chunks (2 token tiles each)
SZ = 512                 # spans per stage-2 tile
NBT = TC // 128          # block tiles = 16
NT = NBT + 1             # + 1 overflow tile
MPAD = NT * SZ           # padded span count per core = 8704
HP2 = 256                # table row elems (fp16 -> 512B)
W1N = 2 * H              # 300: stage-1 moving operand width (A | G)


def build_graph():
    """Build the per-core SPMD Bass graph (fixed shapes, input independent)."""
    from concourse import bacc
    import concourse.mybir as mybir
    from concourse.tile import TileContext
    from concourse import library_config
    from concourse.tile_rust import add_dep_helper

    fp16 = mybir.dt.float16
    f32 = mybir.dt.float32
    i16 = mybir.dt.int16
    u8 = mybir.dt.uint8
    AF = mybir.ActivationFunctionType
    EQ = mybir.AluOpType.is_equal
    ADD = mybir.AluOpType.add

    nc = bacc.Bacc(num_swdge_queues=4)

    from concourse.tile_rust import add_dep_helper as _adh
    _chains = {"sync": [], "scalar": []}

    def _chained(which, eng):
        def dma(**kw):
            g = eng(**kw)
            if _chains[which]:
                _adh(g.ins, _chains[which][-1].ins, False, "pin dma order")
            _chains[which].append(g)
            return g
        return dma

    xsT_d = nc.declare_dram_parameter("xsT", [128, N_CH, N_KB, 256], fp16, isOutput=False)
    wc_d = nc.declare_dram_parameter("wc", [128, N_KB * W1N], fp16, isOutput=False)
    w2p_d = nc.declare_dram_parameter("w2p", [128, 2 * L], fp16, isOutput=False)
    b1p_d = nc.declare_dram_parameter("b1p", [128, 2], f32, isOutput=False)
    b2p_d = nc.declare_dram_parameter("b2p", [L, 1], f32, isOutput=False)
    iota_d = nc.declare_dram_parameter("iota", [128, 1], u8, isOutput=False)
    i0v_d = nc.declare_dram_parameter("i0v", [128, NBT * SZ], u8, isOutput=False)
    idxg_d = nc.declare_dram_parameter("idxg", [128, NT * 32], i16, isOutput=False)
    idxa_d = nc.declare_dram_parameter("idxa", [128, 32], i16, isOutput=False)
    outT_d = nc.declare_dram_parameter("outT", [L, MPAD], f32, isOutput=True)

    sync_dma = _chained("sync", nc.sync.dma_start)
    scalar_dma = _chained("scalar", nc.scalar.dma_start)

    with TileContext(nc) as tc:
        with (
            tc.tile_pool(name="const", bufs=1) as constp,
            tc.tile_pool(name="xst", bufs=1) as xstp,
            tc.tile_pool(name="atile", bufs=1) as atilep,
            tc.tile_pool(name="dram", bufs=1, space="DRAM") as dramp,
            tc.tile_pool(name="s0", bufs=1) as s0p,
            tc.tile_pool(name="gg", bufs=1) as ggp,
            tc.tile_pool(name="psX", bufs=5, space="PSUM") as psXp,
            tc.tile_pool(name="ps2", bufs=3, space="PSUM") as ps2p,
            tc.tile_pool(name="h0", bufs=6) as h0p,
            tc.tile_pool(name="t0", bufs=6) as t0p,
            tc.tile_pool(name="t1", bufs=6) as t1p,
            tc.tile_pool(name="h1", bufs=6) as h1p,
            tc.tile_pool(name="ot", bufs=8) as otp,
        ):
            nc.gpsimd.load_library(library_config.mlp)

            # ---- input loads ----
            # stage-1 weights first: the PE needs them at t~1us. The tile
            # scheduler reorders DMA issues, so pin queue order explicitly.
            # wc rides the sync queue (earliest to start after engine init)
            wc_lo = constp.tile([128, 4 * W1N], fp16)
            wc_hi = constp.tile([128, 4 * W1N], fp16)
            wc_dma = sync_dma(out=wc_lo[:], in_=wc_d[:, 0:4 * W1N])
            # xs^T chunks + small consts on the sync queue
            xst_sb = xstp.tile([128, N_CH, N_KB, 256], fp16)
            xst_dmas = [wc_dma]
            for ch in range(N_CH):
                x = sync_dma(
                    out=xst_sb[:, ch, :, :], in_=xsT_d[:, ch, :, :]
                )
                add_dep_helper(x.ins, xst_dmas[-1].ins, False, "chunk order")
                xst_dmas.append(x)
                if ch == 0:
                    sync_dma(out=wc_hi[:], in_=wc_d[:, 4 * W1N:])
            idxg_sb = constp.tile([128, NT * 32], i16)
            sync_dma(out=idxg_sb[:], in_=idxg_d[:])
            idxa_sb = constp.tile([128, 32], i16)
            sync_dma(out=idxa_sb[:], in_=idxa_d[:])
            iota_sb = constp.tile([128, 1], u8)
            sync_dma(out=iota_sb[:], in_=iota_d[:])
            i0v_sb = constp.tile([128, NBT * SZ], u8)
            sync_dma(out=i0v_sb[:], in_=i0v_d[:])
            w2p_sb = constp.tile([128, 2 * L], fp16)
            c1 = sync_dma(out=w2p_sb[:], in_=w2p_d[:])
            b1p_sb = constp.tile([128, 2], f32)
            c2 = sync_dma(out=b1p_sb[:], in_=b1p_d[:])
            b2p_sb = constp.tile([L, 1], f32)
            c3 = sync_dma(out=b2p_sb[:], in_=b2p_d[:])
            for c in (c1, c2, c3):
                add_dep_helper(c.ins, xst_dmas[1].ins, False, "xst loads first")

            # ---- combined DRAM gather table: token t -> [A row | G row],
            # 512B each; gathers pick a half via base offset + elem_step ----
            tab = dramp.tile([TC, 2, HP2], fp16)

            # ---- A-side one-hot tiles (DVE; fully overlapped with stage 1) ----
            s0_tiles = []
            for b in range(NBT):
                s0 = s0p.tile([128, SZ], fp16, tag=f"s0_{b}", name=f"s0_{b}")
                nc.vector.tensor_tensor(
                    out=s0[:],
                    in0=iota_sb[:, 0:1].to_broadcast([128, SZ]),
                    in1=i0v_sb[:, b * SZ:(b + 1) * SZ],
                    op=EQ,
                )
                s0_tiles.append(s0)

            # ---- stage 1: token tables ----
            # a_tiles hold [A row | G row] per token: one-hot stationaries
            # (A half) and staging for the combined table DMA
            a_tiles = []
            last_tab_dma = None
            for tt in range(N_TT):
                ch, hf = tt // 2, tt % 2
                ps = psXp.tile([128, 512], f32, tag="ps")
                for kb in range(N_KB):
                    wch = wc_lo if kb < 4 else wc_hi
                    nc.tensor.matmul(
                        ps[:, 0:W1N],
                        xst_sb[:, ch, kb, hf * 128:(hf + 1) * 128],
                        wch[:, (kb % 4) * W1N:(kb % 4 + 1) * W1N],
                        start=(kb == 0),
                        stop=(kb == N_KB - 1),
                    )
                at = atilep.tile([128, 2, HP2], fp16, tag=f"a_{tt}", name=f"a_{tt}")
                nc.vector.memset(at[:, 0, H:HP2], 0.0)
                nc.vector.memset(at[:, 1, H:HP2], 0.0)
                nc.scalar.activation(at[:, 0, 0:H], ps[:, 0:H], AF.Copy)
                nc.scalar.activation(at[:, 1, 0:H], ps[:, H:W1N], AF.Copy)
                # same-queue FIFO: the last table write covers all 16
                last_tab_dma = sync_dma(
                    out=tab[tt * 128:(tt + 1) * 128, :, :], in_=at[:]
                )
                a_tiles.append(at)

            # ---- gathers: DRAM-source transpose, 4 SWDGE queues ----
            # Chain the gathers so their scheduled order matches emission
            # order: Tile cycles 8 DMASW lane sems over Pool DMAs in
            # *scheduled* order and each lane sem must stay locked to a
            # single SWDGE queue; with queue = k % 4 and lane = k % 8 the
            # mapping is consistent only if the order is pinned.
            # 512 idxs per gather: >512-idx transpose gathers overflow the
            # 128-entry SWDGE descriptor ring and crash the ucode.
            gg_tiles = []
            prev_g = None
            qn = 0
            for st in range(NT):
                gg = ggp.tile([128, 2, SZ], fp16, tag=f"gg_{st}", name=f"gg_{st}")
                g = nc.gpsimd.dma_gather(
                    gg[:],
                    tab[:, 1, :],
                    idxg_sb[:, st * 32:(st + 1) * 32],
                    SZ,
                    SZ,
                    elem_size=HP2,
                    elem_step=2 * HP2,
                    transpose=True,
                    queue_num=qn % 4,
                )
                qn += 1
                add_dep_helper(g.ins, last_tab_dma.ins, True, "gather after table")
                if prev_g is not None:
                    add_dep_helper(g.ins, prev_g.ins, False, "pin gather order")
                prev_g = g
                gg_tiles.append(gg)

            def gg_lo(st):
                return gg_tiles[st][:, 0, :]

            def gg_hi(st):
                return gg_tiles[st][0:22, 1, :]
            ggA = ggp.tile([128, 2, SZ], fp16, tag="ggA", name="ggA")
            g = nc.gpsimd.dma_gather(
                ggA[:],
                tab[:, 0, :],
                idxa_sb[:],
                SZ,
                SZ,
                elem_size=HP2,
                elem_step=2 * HP2,
                transpose=True,
                queue_num=qn % 4,
            )
            add_dep_helper(g.ins, last_tab_dma.ins, True, "gather after table")
            add_dep_helper(g.ins, prev_g.ins, False, "pin gather order")

            # ---- stage 2 ----
            for st in range(NT):
                t0 = t0p.tile([128, SZ], fp16, tag="t0")
                t1 = t1p.tile([22, SZ], fp16, tag="t1")
                if st < NBT:
                    pA0 = psXp.tile([128, SZ], f32, tag="ps")
                    nc.tensor.matmul(
                        pA0[:], a_tiles[st][:, 0, 0:128], s0_tiles[st][:],
                        start=True, stop=True,
                    )
                    nc.vector.tensor_tensor(
                        out=t0[:], in0=pA0[:], in1=gg_lo(st), op=ADD
                    )
                    pA1 = psXp.tile([22, SZ], f32, tag="ps")
                    nc.tensor.matmul(
                        pA1[:], a_tiles[st][:, 0, 128:H], s0_tiles[st][:],
                        start=True, stop=True,
                    )
                    nc.vector.tensor_tensor(
                        out=t1[:], in0=pA1[:], in1=gg_hi(st), op=ADD
                    )
                else:
                    nc.vector.tensor_tensor(
                        out=t0[:], in0=ggA[:, 0, :], in1=gg_lo(st), op=ADD
                    )
                    nc.vector.tensor_tensor(
                        out=t1[:], in0=ggA[0:22, 1, :], in1=gg_hi(st), op=ADD
                    )
                h0 = h0p.tile([128, SZ], fp16, tag="h0")
                h1 = h1p.tile([22, SZ], fp16, tag="h1")
                nc.scalar.activation(h0[:], t0[:], AF.Relu, bias=b1p_sb[:, 0:1])
                nc.scalar.activation(h1[:], t1[:], AF.Relu, bias=b1p_sb[0:22, 1:2])
                ps2 = ps2p.tile([L, SZ], f32, tag="ps2")
                nc.tensor.matmul(
                    ps2[:], w2p_sb[:, 0:L], h0[:], start=True, stop=False
                )
                nc.tensor.matmul(
                    ps2[:], w2p_sb[0:22, L:2 * L], h1[:], start=False, stop=True
                )
                ot = otp.tile([L, SZ], f32)
                if st % 2 == 0:
                    nc.scalar.activation(ot[:], ps2[:], AF.Identity, bias=b2p_sb[:])
                else:
                    nc.vector.tensor_scalar_add(out=ot[:], in0=ps2[:], scalar1=b2p_sb[:])
                sync_dma(
                    out=outT_d[:, st * SZ:(st + 1) * SZ], in_=ot[:]
                )

    return nc


def _wrap_idx(idx_1d):
    """SWDGE index layout: [16, n/16].T wrapped, replicated to 128 rows."""
    n = len(idx_1d)
    arr16 = idx_1d.reshape(n // 16, 16).T
    return np.tile(arr16, (8, 1)).astype(np.int16)


def prep_inputs(xs, spans, batch_ids, W1, b1, W2, b2):
    """Host-side routing and layout. Returns (in_maps, slot_maps)."""
    xs = np.asarray(xs, dtype=np.float32)
    spans = np.asarray(spans).astype(np.int64)
    batch_ids = np.asarray(batch_ids).astype(np.int64)
    W1 = np.asarray(W1, dtype=np.float32)
    b1 = np.asarray(b1, dtype=np.float32)
    W2 = np.asarray(W2, dtype=np.float32)
    b2 = np.asarray(b2, dtype=np.float32)

    core = batch_ids // BPC
    local0 = (batch_ids % BPC) * T + spans[:, 0]
    local1 = (batch_ids % BPC) * T + spans[:, 1]

    # shared weights
    W1h = W1.astype(np.float16)
    wc = np.empty((128, N_KB * W1N), np.float16)
    for kb in range(N_KB):
        wc[:, kb * W1N:kb * W1N + H] = W1h[kb * 128:(kb + 1) * 128, :]
        wc[:, kb * W1N + H:(kb + 1) * W1N] = W1h[D + kb * 128:D + (kb + 1) * 128, :]
    w2p = np.zeros((128, 2 * L), np.float16)
    w2p[:, 0:L] = W2[0:128].astype(np.float16)
    w2p[0:H - 128, L:2 * L] = W2[128:H].astype(np.float16)
    b1p = np.zeros((128, 2), np.float32)
    b1p[:, 0] = b1[0:128]
    b1p[0:H - 128, 1] = b1[128:H]
    b2p = np.ascontiguousarray(b2.reshape(L, 1))
    iota = np.arange(128, dtype=np.uint8).reshape(128, 1)

    in_maps = []
    slot_maps = []
    for c in range(NCORES):
        sel = np.nonzero(core == c)[0]
        i0 = local0[sel]
        i1 = local1[sel]
        blk = i0 >> 7

        slot_map = np.full(MPAD, -1, np.int64)
        s_i0 = np.zeros(MPAD, np.int64)
        s_i1 = np.zeros(MPAD, np.int64)
        overflow = []
        for b in range(NBT):
            ids = np.nonzero(blk == b)[0]
            # sort by end-token: the G gathers then read the DRAM table
            # mostly in ascending order (better row-buffer locality)
            ids = ids[np.argsort(i1[ids], kind="stable")]
            take = ids[:SZ]
            overflow.append(ids[SZ:])
            base = b * SZ
            ntk = len(take)
            slot_map[base:base + ntk] = sel[take]
            s_i0[base:base + ntk] = i0[take]
            s_i1[base:base + ntk] = i1[take]
            # pad: i0 at block start (one-hot row 0), i1 = 0
            s_i0[base + ntk:base + SZ] = b * 128
        ov = np.concatenate(overflow) if overflow else np.empty(0, np.int64)
        assert len(ov) <= SZ, f"core {c}: overflow {len(ov)} > {SZ}"
        base = NBT * SZ
        nov = len(ov)
        slot_map[base:base + nov] = sel[ov]
        s_i0[base:base + nov] = i0[ov]
        s_i1[base:base + nov] = i1[ov]

        # one-hot compare values: block-relative begin-token, u8
        i0rel = (s_i0[:NBT * SZ] & 127).astype(np.uint8)
        i0v = np.ascontiguousarray(np.broadcast_to(i0rel, (128, NBT * SZ)))
        idxg = _wrap_idx(s_i1)
        idxa = _wrap_idx(s_i0[NBT * SZ:])

        # xsT: [p, ch, kb, t] = xs[token=ch*256+t, d=kb*128+p], fp16
        xs_c = xs[c * BPC:(c + 1) * BPC].reshape(TC, D).astype(np.float16)
        xsT = np.ascontiguousarray(
            xs_c.T.reshape(N_KB, 128, N_CH, 256).transpose(1, 2, 0, 3)
        )

        in_maps.append({
            "xsT": xsT, "wc": wc, "w2p": w2p,
            "b1p": b1p, "b2p": b2p, "iota": iota, "i0v": i0v,
            "idxg": idxg, "idxa": idxa,
        })
        slot_maps.append(slot_map)

    return in_maps, slot_maps


def _scatter_out(results, slot_maps):
    out = np.empty((N_SPANS, L), np.float32)
    for c in range(NCORES):
        sm = slot_maps[c]
        valid = sm >= 0
        out[sm[valid]] = results[c]["outT"].T[valid]
    return out


def _install_ntff_shim():
    """Provide antenv.axon_hooks (missing on this image) so that
    run_bass_kernel_spmd(trace=True) can drive NTFF profiling via the
    axon .so. Only used by the profiling path."""
    import sys
    import types
    import ctypes
    import contextlib

    if "antenv.axon_hooks" in sys.modules:
        return
    import antenv

    holder = {"hook": None}
    mod = types.ModuleType("antenv.axon_hooks")
    mod.set_axon_ntff_profile_hook = lambda h: holder.__setitem__("hook", h)
    mod.get_axon_ntff_profile_hook = lambda: holder["hook"]
    sys.modules["antenv.axon_hooks"] = mod
    antenv.axon_hooks = mod

    so_path = "/opt/axon/libaxon_pjrt.so"
    try:
        lib = ctypes.CDLL(so_path)
    except OSError:
        return
    if not hasattr(lib, "axon_start_nrt_profile"):
        return
    lib.axon_start_nrt_profile.argtypes = [
        ctypes.POINTER(ctypes.c_int64),
        ctypes.c_size_t,
    ]
    lib.axon_start_nrt_profile.restype = ctypes.c_int64
    lib.axon_stop_nrt_profile.argtypes = [ctypes.c_char_p]
    lib.axon_stop_nrt_profile.restype = ctypes.c_int64

    @contextlib.contextmanager
    def _hook(output_dir, device_ids):
        import jax

        jax.devices()
        if device_ids:
            ids = (ctypes.c_int64 * len(device_ids))(*device_ids)
            rc = lib.axon_start_nrt_profile(ids, len(device_ids))
        else:
            rc = lib.axon_start_nrt_profile(None, 0)
        if rc != 0:
            raise RuntimeError(f"axon_start_nrt_profile rc={rc}")
        try:
            yield
        finally:
            n = lib.axon_stop_nrt_profile(str(output_dir).encode())
            print(f"profile: {n} file(s) written to {output_dir}")

    mod.set_axon_ntff_profile_hook(_hook)


def run(inputs: dict, trace: bool = False):
    """Run on the 8 NeuronCores. Returns (out, BassKernelResults)."""
    from concourse import bass_utils
    from concourse.bass_utils import run_bass_kernel_spmd

    if trace:
        _install_ntff_shim()
        bass_utils.upload_artifacts = lambda tmpdir: str(tmpdir)

    in_maps, slot_maps = prep_inputs(**inputs)
    nc = build_graph()
    nc.finalize()
    res = run_bass_kernel_spmd(
        nc, in_maps, list(range(NCORES)), trace=trace
    )
    return _scatter_out(res.results, slot_maps), res


def kernel(**inputs) -> np.ndarray:
    out, _ = run(inputs, trace=False)
    return out



# revision 41
# speedup vs baseline: 1.0299x; 1.0299x over previous
"""Trainium2 Bass kernel for nn_BaselineSpanScorer (span-pair MLP scorer), v3.

reference:
    xs        [32, 512, 1024] f32
    spans     [65536, 2] int   (begin/end token index within sequence)
    batch_ids [65536] int
    W1 [2048, 150], b1 [150], W2 [150, 17], b2 [17]
    out[n] = relu(concat(xs[b, s0], xs[b, s1]) @ W1 + b1) @ W2 + b2

Strategy (8 NeuronCores, data parallel, per-core graph is input-shape fixed):
  - Shard xs by batch: core c owns batches [4c, 4c+4) = 2048 token rows.
  - Factorize: A[t] = xs[t] @ W1[:1024], G[t] = xs[t] @ W1[1024:], so
    pre[n] = A[i0_n] + G[i1_n] + b1 (token reuse cuts matmul work ~4x).
  - Stage 1: 16 token tiles x 8 K-blocks of fused [A|G] 300-col fp16
    matmuls. A tiles stay in SBUF (one-hot lhsT) and are also DMAed to a
    DRAM table (for the overflow gather); G tiles go to a DRAM table.
  - Spans are bucketed by i0 token block (16 buckets of 512 + 1 overflow
    tile): a block tile's one-hot contraction needs exactly ONE 128-token
    window. Overflow spans use a gathered A side instead.
  - G side (and overflow A side) via DRAM-source transpose dma_gather
    (features land on partitions), 4 SWDGE queues.
  - Stage 2 per span tile: PSUM accumulates onehot-A (or identity@gatherA)
    + identity@gatherG on the PE; hi-22 features add on DVE; relu+bias on
    ACT; [150]x[17] W2 contraction; bias; DMA out scores^T.
  - Host scatters per-core outputs back to original span order.

Compute dtype fp16 (rel err ~1e-3 vs f32 reference), f32 output.
"""

import os

os.environ.setdefault("MYCRO_LOCAL_CACHE", "1")

import numpy as np

# ---------------- problem constants (hardcoded per spec) ----------------
B, T, D = 32, 512, 1024
N_SPANS = 65536
H, L = 150, 17
NCORES = 8
BPC = B // NCORES        # batches per core = 4
TC = BPC * T             # tokens per core = 2048
N_KB = D // 128          # K blocks in stage 1 = 8
N_TT = TC // 128         # token tiles in stage 1 = 16
N_CH = 8                 # xsT load chunks (2 token tiles each)
SZ = 512                 # spans per stage-2 tile
NBT = TC // 128          # block tiles = 16
NT = NBT + 1             # + 1 overflow tile
MPAD = NT * SZ           # padded span count per core = 8704
HP2 = 256                # table row elems (fp16 -> 512B)
W1N = 2 * H              # 300: stage-1 moving operand width (A | G)


def build_graph():
    """Build the per-core SPMD Bass graph (fixed shapes, input independent)."""
    from concourse import bacc
    import concourse.mybir as mybir
    from concourse.tile import TileContext
    from concourse import library_config
    from concourse.tile_rust import add_dep_helper

    fp16 = mybir.dt.float16
    f32 = mybir.dt.float32
    i16 = mybir.dt.int16
    u8 = mybir.dt.uint8
    AF = mybir.ActivationFunctionType
    EQ = mybir.AluOpType.is_equal
    ADD = mybir.AluOpType.add

    nc = bacc.Bacc(num_swdge_queues=4)

    from concourse.tile_rust import add_dep_helper as _adh
    _chains = {"sync": [], "scalar": []}

    def _chained(which, eng):
        def dma(**kw):
            g = eng(**kw)
            if _chains[which]:
                _adh(g.ins, _chains[which][-1].ins, False, "pin dma order")
            _chains[which].append(g)
            return g
        return dma

    xsT_d = nc.declare_dram_parameter("xsT", [128, N_CH, N_KB, 256], fp16, isOutput=False)
    wc_d = nc.declare_dram_parameter("wc", [128, N_KB * W1N], fp16, isOutput=False)
    w2p_d = nc.declare_dram_parameter("w2p", [128, 2 * L], fp16, isOutput=False)
    b1p_d = nc.declare_dram_parameter("b1p", [128, 2], f32, isOutput=False)
    b2p_d = nc.declare_dram_parameter("b2p", [L, 1], f32, isOutput=False)
    iota_d = nc.declare_dram_parameter("iota", [128, 1], u8, isOutput=False)
    i0v_d = nc.declare_dram_parameter("i0v", [128, NBT * SZ], u8, isOutput=False)
    idxg_d = nc.declare_dram_parameter("idxg", [128, NT * 32], i16, isOutput=False)
    idxa_d = nc.declare_dram_parameter("idxa", [128, 32], i16, isOutput=False)
    outT_d = nc.declare_dram_parameter("outT", [L, MPAD], f32, isOutput=True)

    sync_dma = _chained("sync", nc.sync.dma_start)
    scalar_dma = _chained("scalar", nc.scalar.dma_start)

    with TileContext(nc) as tc:
        with (
            tc.tile_pool(name="const", bufs=1) as constp,
            tc.tile_pool(name="xst", bufs=1) as xstp,
            tc.tile_pool(name="atile", bufs=1) as atilep,
            tc.tile_pool(name="dram", bufs=1, space="DRAM") as dramp,
            tc.tile_pool(name="s0", bufs=1) as s0p,
            tc.tile_pool(name="gg", bufs=1) as ggp,
            tc.tile_pool(name="psX", bufs=5, space="PSUM") as psXp,
            tc.tile_pool(name="ps2", bufs=3, space="PSUM") as ps2p,
            tc.tile_pool(name="h0", bufs=6) as h0p,
            tc.tile_pool(name="t0", bufs=6) as t0p,
            tc.tile_pool(name="t1", bufs=6) as t1p,
            tc.tile_pool(name="h1", bufs=6) as h1p,
            tc.tile_pool(name="ot", bufs=8) as otp,
        ):
            nc.gpsimd.load_library(library_config.mlp)

            # ---- input loads ----
            # stage-1 weights first: the PE needs them at t~1us. The tile
            # scheduler reorders DMA issues, so pin queue order explicitly.
            # wc rides the sync queue (earliest to start after engine init)
            wc_lo = constp.tile([128, 4 * W1N], fp16)
            wc_hi = constp.tile([128, 4 * W1N], fp16)
            wc_dma = sync_dma(out=wc_lo[:], in_=wc_d[:, 0:4 * W1N])
            # xs^T chunks + small consts on the sync queue
            xst_sb = xstp.tile([128, N_CH, N_KB, 256], fp16)
            xst_dmas = [wc_dma]
            for ch in range(N_CH):
                x = sync_dma(
                    out=xst_sb[:, ch, :, :], in_=xsT_d[:, ch, :, :]
                )
                add_dep_helper(x.ins, xst_dmas[-1].ins, False, "chunk order")
                xst_dmas.append(x)
                if ch == 0:
                    sync_dma(out=wc_hi[:], in_=wc_d[:, 4 * W1N:])
            idxg_sb = constp.tile([128, NT * 32], i16)
            sync_dma(out=idxg_sb[:], in_=idxg_d[:])
            idxa_sb = constp.tile([128, 32], i16)
            sync_dma(out=idxa_sb[:], in_=idxa_d[:])
            iota_sb = constp.tile([128, 1], u8)
            sync_dma(out=iota_sb[:], in_=iota_d[:])
            i0v_sb = constp.tile([128, NBT * SZ], u8)
            sync_dma(out=i0v_sb[:], in_=i0v_d[:])
            w2p_sb = constp.tile([128, 2 * L], fp16)
            c1 = sync_dma(out=w2p_sb[:], in_=w2p_d[:])
            b1p_sb = constp.tile([128, 2], f32)
            c2 = sync_dma(out=b1p_sb[:], in_=b1p_d[:])
            b2p_sb = constp.tile([L, 1], f32)
            c3 = sync_dma(out=b2p_sb[:], in_=b2p_d[:])
            for c in (c1, c2, c3):
                add_dep_helper(c.ins, xst_dmas[1].ins, False, "xst loads first")

            # ---- combined DRAM gather table: token t -> [A row | G row],
            # 512B each; gathers pick a half via base offset + elem_step ----
            tab = dramp.tile([TC, 2, HP2], fp16)

            # ---- A-side one-hot tiles (DVE; fully overlapped with stage 1) ----
            s0_tiles = []
            for b in range(NBT):
                s0 = s0p.tile([128, SZ], fp16, tag=f"s0_{b}", name=f"s0_{b}")
                nc.vector.tensor_tensor(
                    out=s0[:],
                    in0=iota_sb[:, 0:1].to_broadcast([128, SZ]),
                    in1=i0v_sb[:, b * SZ:(b + 1) * SZ],
                    op=EQ,
                )
                s0_tiles.append(s0)

            # ---- stage 1: token tables ----
            # a_tiles hold [A row | G row] per token: one-hot stationaries
            # (A half) and staging for the combined table DMA
            a_tiles = []
            last_tab_dma = None
            for tt in range(N_TT):
                ch, hf = tt // 2, tt % 2
                ps = psXp.tile([128, 512], f32, tag="ps")
                for kb in range(N_KB):
                    wch = wc_lo if kb < 4 else wc_hi
                    nc.tensor.matmul(
                        ps[:, 0:W1N],
                        xst_sb[:, ch, kb, hf * 128:(hf + 1) * 128],
                        wch[:, (kb % 4) * W1N:(kb % 4 + 1) * W1N],
                        start=(kb == 0),
                        stop=(kb == N_KB - 1),
                    )
                at = atilep.tile([128, 2, HP2], fp16, tag=f"a_{tt}", name=f"a_{tt}")
                nc.vector.memset(at[:, 0, H:HP2], 0.0)
                nc.vector.memset(at[:, 1, H:HP2], 0.0)
                nc.scalar.activation(at[:, 0, 0:H], ps[:, 0:H], AF.Copy)
                nc.scalar.activation(at[:, 1, 0:H], ps[:, H:W1N], AF.Copy)
                # same-queue FIFO: the last table write covers all 16
                last_tab_dma = sync_dma(
                    out=tab[tt * 128:(tt + 1) * 128, :, :], in_=at[:]
                )
                a_tiles.append(at)

            # ---- gathers: DRAM-source transpose, 4 SWDGE queues ----
            # Chain the gathers so their scheduled order matches emission
            # order: Tile cycles 8 DMASW lane sems over Pool DMAs in
            # *scheduled* order and each lane sem must stay locked to a
            # single SWDGE queue; with queue = k % 4 and lane = k % 8 the
            # mapping is consistent only if the order is pinned.
            # 512 idxs per gather: >512-idx transpose gathers overflow the
            # 128-entry SWDGE descriptor ring and crash the ucode.
            gg_tiles = []
            prev_g = None
            qn = 0
            for st in range(NT):
                gg = ggp.tile([128, 2, SZ], fp16, tag=f"gg_{st}", name=f"gg_{st}")
                g = nc.gpsimd.dma_gather(
                    gg[:],
                    tab[:, 1, :],
                    idxg_sb[:, st * 32:(st + 1) * 32],
                    SZ,
                    SZ,
                    elem_size=HP2,
                    elem_step=2 * HP2,
                    transpose=True,
                    queue_num=qn % 4,
                )
                qn += 1
                add_dep_helper(g.ins, last_tab_dma.ins, True, "gather after table")
                if prev_g is not None:
                    add_dep_helper(g.ins, prev_g.ins, False, "pin gather order")
                prev_g = g
                gg_tiles.append(gg)

            def gg_lo(st):
                return gg_tiles[st][:, 0, :]

            def gg_hi(st):
                return gg_tiles[st][0:22, 1, :]
            ggA = ggp.tile([128, 2, SZ], fp16, tag="ggA", name="ggA")
            g = nc.gpsimd.dma_gather(
                ggA[:],
                tab[:, 0, :],
                idxa_sb[:],
                SZ,
                SZ,
                elem_size=HP2,
                elem_step=2 * HP2,
                transpose=True,
                queue_num=qn % 4,
            )
            add_dep_helper(g.ins, last_tab_dma.ins, True, "gather after table")
            add_dep_helper(g.ins, prev_g.ins, False, "pin gather order")

            # ---- stage 2 ----
            for st in range(NT):
                t0 = t0p.tile([128, SZ], fp16, tag="t0")
                t1 = t1p.tile([22, SZ], fp16, tag="t1")
                if st < NBT:
                    pA0 = psXp.tile([128, SZ], f32, tag="ps")
                    nc.tensor.matmul(
                        pA0[:], a_tiles[st][:, 0, 0:128], s0_tiles[st][:],
                        start=True, stop=True,
                    )
                    nc.vector.tensor_tensor(
                        out=t0[:], in0=pA0[:], in1=gg_lo(st), op=ADD
                    )
                    pA1 = psXp.tile([22, SZ], f32, tag="ps")
                    nc.tensor.matmul(
                        pA1[:], a_tiles[st][:, 0, 128:H], s0_tiles[st][:],
                        start=True, stop=True,
                    )
                    nc.vector.tensor_tensor(
                        out=t1[:], in0=pA1[:], in1=gg_hi(st), op=ADD
                    )
                else:
                    nc.vector.tensor_tensor(
                        out=t0[:], in0=ggA[:, 0, :], in1=gg_lo(st), op=ADD
                    )
                    nc.vector.tensor_tensor(
                        out=t1[:], in0=ggA[0:22, 1, :], in1=gg_hi(st), op=ADD
                    )
                h0 = h0p.tile([128, SZ], fp16, tag="h0")
                h1 = h1p.tile([22, SZ], fp16, tag="h1")
                nc.scalar.activation(h0[:], t0[:], AF.Relu, bias=b1p_sb[:, 0:1])
                nc.scalar.activation(h1[:], t1[:], AF.Relu, bias=b1p_sb[0:22, 1:2])
                ps2 = ps2p.tile([L, SZ], f32, tag="ps2")
                nc.tensor.matmul(
                    ps2[:], w2p_sb[:, 0:L], h0[:], start=True, stop=False
                )
                nc.tensor.matmul(
                    ps2[:], w2p_sb[0:22, L:2 * L], h1[:], start=False, stop=True
                )
                ot = otp.tile([L, SZ], f32)
                if st % 2 == 0:
                    nc.scalar.activation(ot[:], ps2[:], AF.Identity, bias=b2p_sb[:])
                else:
                    nc.vector.tensor_scalar_add(out=ot[:], in0=ps2[:], scalar1=b2p_sb[:])
                sync_dma(
                    out=outT_d[:, st * SZ:(st + 1) * SZ], in_=ot[:]
                )

    return nc


def _wrap_idx(idx_1d):
    """SWDGE index layout: [16, n/16].T wrapped, replicated to 128 rows."""
    n = len(idx_1d)
    arr16 = idx_1d.reshape(n // 16, 16).T
    return np.tile(arr16, (8, 1)).astype(np.int16)


def prep_inputs(xs, spans, batch_ids, W1, b1, W2, b2):
    """Host-side routing and layout. Returns (in_maps, slot_maps)."""
    xs = np.asarray(xs, dtype=np.float32)
    spans = np.asarray(spans).astype(np.int64)
    batch_ids = np.asarray(batch_ids).astype(np.int64)
    W1 = np.asarray(W1, dtype=np.float32)
    b1 = np.asarray(b1, dtype=np.float32)
    W2 = np.asarray(W2, dtype=np.float32)
    b2 = np.asarray(b2, dtype=np.float32)

    core = batch_ids // BPC
    local0 = (batch_ids % BPC) * T + spans[:, 0]
    local1 = (batch_ids % BPC) * T + spans[:, 1]

    # shared weights
    W1h = W1.astype(np.float16)
    wc = np.empty((128, N_KB * W1N), np.float16)
    for kb in range(N_KB):
        wc[:, kb * W1N:kb * W1N + H] = W1h[kb * 128:(kb + 1) * 128, :]
        wc[:, kb * W1N + H:(kb + 1) * W1N] = W1h[D + kb * 128:D + (kb + 1) * 128, :]
    w2p = np.zeros((128, 2 * L), np.float16)
    w2p[:, 0:L] = W2[0:128].astype(np.float16)
    w2p[0:H - 128, L:2 * L] = W2[128:H].astype(np.float16)
    b1p = np.zeros((128, 2), np.float32)
    b1p[:, 0] = b1[0:128]
    b1p[0:H - 128, 1] = b1[128:H]
    b2p = np.ascontiguousarray(b2.reshape(L, 1))
    iota = np.arange(128, dtype=np.uint8).reshape(128, 1)

    in_maps = []
    slot_maps = []
    for c in range(NCORES):
        sel = np.nonzero(core == c)[0]
        i0 = local0[sel]
        i1 = local1[sel]
        blk = i0 >> 7

        slot_map = np.full(MPAD, -1, np.int64)
        s_i0 = np.zeros(MPAD, np.int64)
        s_i1 = np.zeros(MPAD, np.int64)
        overflow = []
        for b in range(NBT):
            ids = np.nonzero(blk == b)[0]
            # sort by end-token: the G gathers then read the DRAM table
            # mostly in ascending order (better row-buffer locality)
            ids = ids[np.argsort(i1[ids], kind="stable")]
            take = ids[:SZ]
            overflow.append(ids[SZ:])
            base = b * SZ
            ntk = len(take)
            slot_map[base:base + ntk] = sel[take]
            s_i0[base:base + ntk] = i0[take]
            s_i1[base:base + ntk] = i1[take]
            # pad: i0 at block start (one-hot row 0), i1 = 0
            s_i0[base + ntk:base + SZ] = b * 128
        ov = np.concatenate(overflow) if overflow else np.empty(0, np.int64)
        ov = ov[np.argsort(i1[ov], kind="stable")]
        assert len(ov) <= SZ, f"core {c}: overflow {len(ov)} > {SZ}"
        base = NBT * SZ
        nov = len(ov)
        slot_map[base:base + nov] = sel[ov]
        s_i0[base:base + nov] = i0[ov]
        s_i1[base:base + nov] = i1[ov]

        # one-hot compare values: block-relative begin-token, u8
        i0rel = (s_i0[:NBT * SZ] & 127).astype(np.uint8)
        i0v = np.ascontiguousarray(np.broadcast_to(i0rel, (128, NBT * SZ)))
        idxg = _wrap_idx(s_i1)
        idxa = _wrap_idx(s_i0[NBT * SZ:])

        # xsT: [p, ch, kb, t] = xs[token=ch*256+t, d=kb*128+p], fp16
        xs_c = xs[c * BPC:(c + 1) * BPC].reshape(TC, D).astype(np.float16)
        xsT = np.ascontiguousarray(
            xs_c.T.reshape(N_KB, 128, N_CH, 256).transpose(1, 2, 0, 3)
        )

        in_maps.append({
            "xsT": xsT, "wc": wc, "w2p": w2p,
            "b1p": b1p, "b2p": b2p, "iota": iota, "i0v": i0v,
            "idxg": idxg, "idxa": idxa,
        })
        slot_maps.append(slot_map)

    return in_maps, slot_maps


def _scatter_out(results, slot_maps):
    out = np.empty((N_SPANS, L), np.float32)
    for c in range(NCORES):
        sm = slot_maps[c]
        valid = sm >= 0
        out[sm[valid]] = results[c]["outT"].T[valid]
    return out


def _install_ntff_shim():
    """Provide antenv.axon_hooks (missing on this image) so that
    run_bass_kernel_spmd(trace=True) can drive NTFF profiling via the
    axon .so. Only used by the profiling path."""
    import sys
    import types
    import ctypes
    import contextlib

    if "antenv.axon_hooks" in sys.modules:
        return
    import antenv

    holder = {"hook": None}
    mod = types.ModuleType("antenv.axon_hooks")
    mod.set_axon_ntff_profile_hook = lambda h: holder.__setitem__("hook", h)
    mod.get_axon_ntff_profile_hook = lambda: holder["hook"]
    sys.modules["antenv.axon_hooks"] = mod
    antenv.axon_hooks = mod

    so_path = "/opt/axon/libaxon_pjrt.so"
    try:
        lib = ctypes.CDLL(so_path)
    except OSError:
        return
    if not hasattr(lib, "axon_start_nrt_profile"):
        return
    lib.axon_start_nrt_profile.argtypes = [
        ctypes.POINTER(ctypes.c_int64),
        ctypes.c_size_t,
    ]
    lib.axon_start_nrt_profile.restype = ctypes.c_int64
    lib.axon_stop_nrt_profile.argtypes = [ctypes.c_char_p]
    lib.axon_stop_nrt_profile.restype = ctypes.c_int64

    @contextlib.contextmanager
    def _hook(output_dir, device_ids):
        import jax

        jax.devices()
        if device_ids:
            ids = (ctypes.c_int64 * len(device_ids))(*device_ids)
            rc = lib.axon_start_nrt_profile(ids, len(device_ids))
        else:
            rc = lib.axon_start_nrt_profile(None, 0)
        if rc != 0:
            raise RuntimeError(f"axon_start_nrt_profile rc={rc}")
        try:
            yield
        finally:
            n = lib.axon_stop_nrt_profile(str(output_dir).encode())
            print(f"profile: {n} file(s) written to {output_dir}")

    mod.set_axon_ntff_profile_hook(_hook)


def run(inputs: dict, trace: bool = False):
    """Run on the 8 NeuronCores. Returns (out, BassKernelResults)."""
    from concourse import bass_utils
    from concourse.bass_utils import run_bass_kernel_spmd

    if trace:
        _install_ntff_shim()
        bass_utils.upload_artifacts = lambda tmpdir: str(tmpdir)

    in_maps, slot_maps = prep_inputs(**inputs)
    nc = build_graph()
    nc.finalize()
    res = run_bass_kernel_spmd(
        nc, in_maps, list(range(NCORES)), trace=trace
    )
    return _scatter_out(res.results, slot_maps), res


def kernel(**inputs) -> np.ndarray:
    out, _ = run(inputs, trace=False)
    return out

